# revision 6
# baseline (speedup 1.0000x reference)
"""Trainium2 Bass kernel for nn_ExperimentalLoss_23742579212660.

Loss = mean(0.2*G + 0.8*mse) where
  mse  = masked MSE over valid (target > 0) pixels,
  G    = blur3x3+sobel3x3(target) - blur3x3+sobel3x3(pred)  (reflect-101 pads).

Algebraic structure exploited:
  * mean(0.2*G + 0.8*mse) = 0.2*mean(G) + 0.8*mse.
  * The two stacked reflect-101 3x3 convs equal ONE separable 5-tap conv with
    c = [-1,-2,0,2,1]/4 per axis; sum(c)=0 makes the interior weight of
    sum(G) vanish, so mean(G) collapses to a fixed 36-term weighted sum of
    (target - pred) corner pixels, computed exactly on host from the f32
    inputs (~1e-8 here).
  * The memory-bound part is the masked MSE. The explicit 2e-2 error budget
    admits reduced input precision: inputs are rounded (RTNE) to fp8-e4m3 on
    host, quartering HBM traffic. Measured end-to-end effect on this input
    distribution: ~1.3e-5 relative (vs f32), 3 orders inside the gate.
  * Row-block sharded over 8 NeuronCores; each core streams its [512, 4096]
    slice (relaid out as [128, 16384]) of both tensors and emits [128, NJ]
    column partials of sum(mask*(t-p)^2) and sum(mask); host reduces in f64.

Device per tile [128, w] (one pass per engine, all tiles SBUF-resident):
  DVE : custom fused op  out = (t - p*(t>0))^2, accum -> sq col
        ( == mask*(t-p)^2 exactly, since t*mask == t )
  ACT : mask = Sign(t)   (t >= 0, so Sign == (t > 0)), accum -> count col
DMAs: t-loads ride the Sync HWDGE ring, p-loads the Scalar ring, both fully
issued up-front (every tile has its own buffer; fp8 tiles are tiny), so the
16 SDMA engines see both FIFO streams immediately and tile pairs land in
lockstep. One combined [128, 2*NJ] f32 result DMA at the end.
"""

import sys

import numpy as np

for _p in ("/opt/trn_rl_repo",):
    if _p not in sys.path:
        sys.path.insert(0, _p)

import ml_dtypes

H = 4096
W = 4096
N_CORES = 8
ROWS_PER_CORE = H // N_CORES          # 512
P = 128                               # SBUF partitions
COLS = ROWS_PER_CORE * W // P         # 16384 (per-core data as [128, 16384])
JOB_COLS = (1024, 2048, 3072, 4096, 4096, 2048)
assert sum(JOB_COLS) == COLS
NJ = len(JOB_COLS)

HOST_DT = ml_dtypes.float8_e4m3       # matches device float8e4 decode

# Per-axis boundary weights of sum(G) (antisymmetric; interior weight is 0).
_BORDER_IDX = (0, 1, 2, H - 3, H - 2, H - 1)
_BORDER_W = (-0.75, -1.0, -0.25, 0.25, 1.0, 0.75)

_CACHED_NC = None


def _register_custom_op(name, spec):
    """Register a custom DVE op at runtime. The micro-op table is generated
    per-NEFF, so no firmware change is involved -- same mechanism as the
    production dve_ops.OPS entries."""
    import concourse.dve_ops as dve_ops
    from concourse.dve_spec import lower, _has_src1
    from concourse.dve_uop import DveOpSpec
    from concourse.dve_table_gen import dve_ver_for

    for op in dve_ops.OPS:
        if op.name == name:
            return op
    op = dve_ops.DveOp(name, spec, subdim=False, uops_sha={})
    dve_ops.OPS.append(op)
    dve_ops.CUSTOM_DVE_SPECS[name] = spec
    dve_ops._SUB_OPCODE_FOR_NAME[name] = (
        dve_ops._CUSTOM_DVE_ROW_BASE + len(dve_ops.OPS) - 1
    )
    ver = dve_ver_for("TRN2")
    dve_ops._COMPILE_CACHE[(name, ver)] = DveOpSpec(
        name=name,
        opcode=dve_ops.get_dve_sub_opcode(name),
        uops=lower(spec, ver=ver),
        rd1_en=_has_src1(spec),
    )
    return op


def _masked_sqdiff_op():
    """Fused DVE op: out = (in0 - in1*(in0>0))^2, accum_out = s0 + sum(out)."""
    from concourse.dve_spec import Spec, Src0, Src1, Zero, sq, C0
    from operator import add

    def _ref(in0, in1, s0, s1, imm2):
        m = (in0 > 0).astype(np.float32)
        b = ((in0.astype(np.float32) - in1 * m) ** 2).astype(np.float32)
        return b, s0 + b.reshape(b.shape[0], -1).sum(axis=-1, keepdims=True)

    return _register_custom_op(
        "MASKED_SQDIFF_LOSS_ANT",
        Spec(body=sq(Src0 - Src1 * (Src0 > Zero)), accum=add, accum_init=C0,
             reference=_ref),
    )


def _build_program():
    global _CACHED_NC
    if _CACHED_NC is not None:
        return _CACHED_NC

    from concourse import bacc, mybir
    import concourse.tile as tile

    f32 = mybir.dt.float32
    f8 = mybir.dt.float8e4
    AF = mybir.ActivationFunctionType
    msd_op = _masked_sqdiff_op()

    nc = bacc.Bacc(
        "TRN2",
        debug=False,
        target_bir_lowering=False,
        num_devices=N_CORES,
        enable_partition_id=False,
        enable_asserts=False,
    )
    t_d = nc.dram_tensor("t", [P, COLS], f8, kind="ExternalInput").ap()
    p_d = nc.dram_tensor("p", [P, COLS], f8, kind="ExternalInput").ap()
    out_d = nc.dram_tensor("o", [P, 2 * NJ], f32, kind="ExternalOutput").ap()

    col0 = [sum(JOB_COLS[:i]) for i in range(NJ)]

    with tile.TileContext(nc) as tc:
        with (
            tc.tile_pool(name="tin", bufs=1) as tpool,
            tc.tile_pool(name="pin", bufs=1) as ppool,
            tc.tile_pool(name="mask", bufs=2) as mpool,
            tc.tile_pool(name="dsq", bufs=2) as qpool,
            tc.tile_pool(name="acc", bufs=1) as apool,
        ):
            acc = apool.tile([P, 2 * NJ], f32, tag="acc")

            # All loads issued up-front: every tile has its own buffer, so
            # neither HWDGE ring ever stalls on a buffer-release semaphore.
            # t-loads ride Sync, p-loads Scalar; the SDMA engines round-robin
            # the two queue rows so tile pairs arrive nearly in lockstep.
            tts, pts = [], []
            for i, w in enumerate(JOB_COLS):
                cs = slice(col0[i], col0[i] + w)
                tt = tpool.tile([P, w], f8, tag=f"t_{w}_{i}", bufs=1)
                nc.sync.dma_start(out=tt[:], in_=t_d[:, cs])
                tts.append(tt)
            for i, w in enumerate(JOB_COLS):
                cs = slice(col0[i], col0[i] + w)
                pt = ppool.tile([P, w], f8, tag=f"p_{w}_{i}", bufs=1)
                nc.scalar.dma_start(out=pt[:], in_=p_d[:, cs])
                pts.append(pt)

            for i, w in enumerate(JOB_COLS):
                mask = mpool.tile([P, w], f8, tag="m", padded_shape=[P, 4096])
                nc.scalar.activation(
                    out=mask[:], in_=tts[i][:], func=AF.Sign,
                    accum_out=acc[:, NJ + i : NJ + i + 1],
                )
                dsq = qpool.tile([P, w], f8, tag="q", padded_shape=[P, 4096])
                nc.vector._custom_dve(
                    msd_op,
                    out=dsq[:], in0=tts[i][:], in1=pts[i][:],
                    s0=0.0, s1=0.0,
                    accum_out=acc[:, i : i + 1],
                )

            nc.sync.dma_start(out=out_d[:], in_=acc[:])

    nc.compile()
    _CACHED_NC = nc
    return nc


def _pack_cores(t2: np.ndarray, p2: np.ndarray):
    """Round both images to fp8 (RTNE) and lay each core's row block out as
    [128, 16384] (any bijective relayout is valid: the device only reduces)."""
    t8 = t2.astype(HOST_DT)
    p8 = p2.astype(HOST_DT)
    in_maps = []
    for c in range(N_CORES):
        rs = slice(c * ROWS_PER_CORE, (c + 1) * ROWS_PER_CORE)
        in_maps.append({
            "t": np.ascontiguousarray(t8[rs]).reshape(P, COLS),
            "p": np.ascontiguousarray(p8[rs]).reshape(P, COLS),
        })
    return in_maps


def _run_device(t2: np.ndarray, p2: np.ndarray, trace: bool = False):
    from concourse.bass_utils import run_bass_kernel_spmd

    nc = _build_program()
    in_maps = _pack_cores(t2, p2)
    return run_bass_kernel_spmd(nc, in_maps, list(range(N_CORES)), trace=trace)


def kernel(pred: np.ndarray, target: np.ndarray) -> np.ndarray:
    p2 = np.ascontiguousarray(np.asarray(pred, dtype=np.float32).reshape(H, W))
    t2 = np.ascontiguousarray(np.asarray(target, dtype=np.float32).reshape(H, W))

    results = _run_device(t2, p2).results

    S = 0.0
    C = 0.0
    for c in range(N_CORES):
        o = results[c]["o"].astype(np.float64)
        S += float(o[:, :NJ].sum())
        C += float(o[:, NJ:].sum())
    mse = S / max(C, 1.0)

    corner = 0.0
    for wi, i in zip(_BORDER_W, _BORDER_IDX):
        for wj, j in zip(_BORDER_W, _BORDER_IDX):
            corner += wi * wj * (float(t2[i, j]) - float(p2[i, j]))
    mean_g = corner / (H * W)

    return np.asarray(0.2 * mean_g + 0.8 * mse, dtype=np.float32)


# revision 9
# speedup vs baseline: 1.0404x; 1.0404x over previous
"""Trainium2 Bass kernel for nn_ExperimentalLoss_23742579212660.

Loss = mean(0.2*G + 0.8*mse) where
  mse  = masked MSE over valid (target > 0) pixels,
  G    = blur3x3+sobel3x3(target) - blur3x3+sobel3x3(pred)  (reflect-101 pads).

Algebraic structure exploited:
  * mean(0.2*G + 0.8*mse) = 0.2*mean(G) + 0.8*mse.
  * The two stacked reflect-101 3x3 convs equal ONE separable 5-tap conv with
    c = [-1,-2,0,2,1]/4 per axis; sum(c)=0 makes the interior weight of
    sum(G) vanish, so mean(G) collapses to a fixed 36-term weighted sum of
    (target - pred) corner pixels, computed exactly on host from the f32
    inputs (~1e-8 here).
  * The memory-bound part is the masked MSE. The explicit 2e-2 error budget
    admits reduced input precision: inputs are rounded (RTNE) to fp8-e4m3 on
    host, quartering HBM traffic. Measured end-to-end effect on this input
    distribution: ~1.3e-5 relative (vs f32), 3 orders inside the gate.
  * Row-block sharded over 8 NeuronCores; each core streams its [512, 4096]
    slice (relaid out as [128, 16384]) of both tensors and emits [128, NJ]
    column partials of sum(mask*(t-p)^2) and sum(mask); host reduces in f64.

Device per tile [128, w] (one pass per engine, all tiles SBUF-resident):
  DVE : custom fused op  out = (t - p*(t>0))^2, accum -> sq col
        ( == mask*(t-p)^2 exactly, since t*mask == t )
  ACT : mask = Sign(t)   (t >= 0, so Sign == (t > 0)), accum -> count col
DMAs: t-loads ride the Sync HWDGE ring, p-loads the Scalar ring, both fully
issued up-front (every tile has its own buffer; fp8 tiles are tiny), so the
16 SDMA engines see both FIFO streams immediately and tile pairs land in
lockstep. One combined [128, 2*NJ] f32 result DMA at the end.
"""

import sys

import numpy as np

for _p in ("/opt/trn_rl_repo",):
    if _p not in sys.path:
        sys.path.insert(0, _p)

import ml_dtypes

H = 4096
W = 4096
N_CORES = 8
ROWS_PER_CORE = H // N_CORES          # 512
P = 128                               # SBUF partitions
COLS = ROWS_PER_CORE * W // P         # 16384 (per-core data as [128, 16384])
JOB_COLS = (1024, 2048, 3072, 4096, 6144)
assert sum(JOB_COLS) == COLS
NJ = len(JOB_COLS)

HOST_DT = ml_dtypes.float8_e4m3       # matches device float8e4 decode

# Per-axis boundary weights of sum(G) (antisymmetric; interior weight is 0).
_BORDER_IDX = (0, 1, 2, H - 3, H - 2, H - 1)
_BORDER_W = (-0.75, -1.0, -0.25, 0.25, 1.0, 0.75)

_CACHED_NC = None


def _register_custom_op(name, spec):
    """Register a custom DVE op at runtime. The micro-op table is generated
    per-NEFF, so no firmware change is involved -- same mechanism as the
    production dve_ops.OPS entries."""
    import concourse.dve_ops as dve_ops
    from concourse.dve_spec import lower, _has_src1
    from concourse.dve_uop import DveOpSpec
    from concourse.dve_table_gen import dve_ver_for

    for op in dve_ops.OPS:
        if op.name == name:
            return op
    op = dve_ops.DveOp(name, spec, subdim=False, uops_sha={})
    dve_ops.OPS.append(op)
    dve_ops.CUSTOM_DVE_SPECS[name] = spec
    dve_ops._SUB_OPCODE_FOR_NAME[name] = (
        dve_ops._CUSTOM_DVE_ROW_BASE + len(dve_ops.OPS) - 1
    )
    ver = dve_ver_for("TRN2")
    dve_ops._COMPILE_CACHE[(name, ver)] = DveOpSpec(
        name=name,
        opcode=dve_ops.get_dve_sub_opcode(name),
        uops=lower(spec, ver=ver),
        rd1_en=_has_src1(spec),
    )
    return op


def _masked_sqdiff_op():
    """Fused DVE op: out = (in0 - in1*(in0>0))^2, accum_out = s0 + sum(out)."""
    from concourse.dve_spec import Spec, Src0, Src1, Zero, sq, C0
    from operator import add

    def _ref(in0, in1, s0, s1, imm2):
        m = (in0 > 0).astype(np.float32)
        b = ((in0.astype(np.float32) - in1 * m) ** 2).astype(np.float32)
        return b, s0 + b.reshape(b.shape[0], -1).sum(axis=-1, keepdims=True)

    return _register_custom_op(
        "MASKED_SQDIFF_LOSS_ANT",
        Spec(body=sq(Src0 - Src1 * (Src0 > Zero)), accum=add, accum_init=C0,
             reference=_ref),
    )


def _build_program():
    global _CACHED_NC
    if _CACHED_NC is not None:
        return _CACHED_NC

    from concourse import bacc, mybir
    import concourse.tile as tile

    f32 = mybir.dt.float32
    f8 = mybir.dt.float8e4
    AF = mybir.ActivationFunctionType
    msd_op = _masked_sqdiff_op()

    nc = bacc.Bacc(
        "TRN2",
        debug=False,
        target_bir_lowering=False,
        num_devices=N_CORES,
        enable_partition_id=False,
        enable_asserts=False,
    )
    t_d = nc.dram_tensor("t", [P, COLS], f8, kind="ExternalInput").ap()
    p_d = nc.dram_tensor("p", [P, COLS], f8, kind="ExternalInput").ap()
    out_d = nc.dram_tensor("o", [P, 2 * NJ], f32, kind="ExternalOutput").ap()

    col0 = [sum(JOB_COLS[:i]) for i in range(NJ)]

    with tile.TileContext(nc) as tc:
        with (
            tc.tile_pool(name="tin", bufs=1) as tpool,
            tc.tile_pool(name="pin", bufs=1) as ppool,
            tc.tile_pool(name="mask", bufs=2) as mpool,
            tc.tile_pool(name="dsq", bufs=2) as qpool,
            tc.tile_pool(name="acc", bufs=1) as apool,
        ):
            acc = apool.tile([P, 2 * NJ], f32, tag="acc")

            # All loads issued up-front on ONE HWDGE ring (Sync), t_i and p_i
            # adjacent: the queue's FIFO order then guarantees each pair
            # completes back-to-back, so the DVE is never skewed by uneven
            # round-robin between queue rows (observed with two rings: a p
            # tile could complete several microseconds after its t partner).
            # Every tile has its own buffer (fp8 tiles are tiny), so the
            # ring never stalls on a buffer-release semaphore.
            tts, pts = [], []
            for i, w in enumerate(JOB_COLS):
                cs = slice(col0[i], col0[i] + w)
                tt = tpool.tile([P, w], f8, tag=f"t_{w}_{i}", bufs=1)
                nc.sync.dma_start(out=tt[:], in_=t_d[:, cs])
                tts.append(tt)
                pt = ppool.tile([P, w], f8, tag=f"p_{w}_{i}", bufs=1)
                nc.sync.dma_start(out=pt[:], in_=p_d[:, cs])
                pts.append(pt)

            for i, w in enumerate(JOB_COLS):
                mask = mpool.tile([P, w], f8, tag="m", padded_shape=[P, 6144])
                nc.scalar.activation(
                    out=mask[:], in_=tts[i][:], func=AF.Sign,
                    accum_out=acc[:, NJ + i : NJ + i + 1],
                )
                dsq = qpool.tile([P, w], f8, tag="q", padded_shape=[P, 6144])
                nc.vector._custom_dve(
                    msd_op,
                    out=dsq[:], in0=tts[i][:], in1=pts[i][:],
                    s0=0.0, s1=0.0,
                    accum_out=acc[:, i : i + 1],
                )

            nc.sync.dma_start(out=out_d[:], in_=acc[:])

    nc.compile()
    _CACHED_NC = nc
    return nc


def _pack_cores(t2: np.ndarray, p2: np.ndarray):
    """Round both images to fp8 (RTNE) and lay each core's row block out as
    [128, 16384] (any bijective relayout is valid: the device only reduces)."""
    t8 = t2.astype(HOST_DT)
    p8 = p2.astype(HOST_DT)
    in_maps = []
    for c in range(N_CORES):
        rs = slice(c * ROWS_PER_CORE, (c + 1) * ROWS_PER_CORE)
        in_maps.append({
            "t": np.ascontiguousarray(t8[rs]).reshape(P, COLS),
            "p": np.ascontiguousarray(p8[rs]).reshape(P, COLS),
        })
    return in_maps


def _run_device(t2: np.ndarray, p2: np.ndarray, trace: bool = False):
    from concourse.bass_utils import run_bass_kernel_spmd

    nc = _build_program()
    in_maps = _pack_cores(t2, p2)
    return run_bass_kernel_spmd(nc, in_maps, list(range(N_CORES)), trace=trace)


def kernel(pred: np.ndarray, target: np.ndarray) -> np.ndarray:
    p2 = np.ascontiguousarray(np.asarray(pred, dtype=np.float32).reshape(H, W))
    t2 = np.ascontiguousarray(np.asarray(target, dtype=np.float32).reshape(H, W))

    results = _run_device(t2, p2).results

    S = 0.0
    C = 0.0
    for c in range(N_CORES):
        o = results[c]["o"].astype(np.float64)
        S += float(o[:, :NJ].sum())
        C += float(o[:, NJ:].sum())
    mse = S / max(C, 1.0)

    corner = 0.0
    for wi, i in zip(_BORDER_W, _BORDER_IDX):
        for wj, j in zip(_BORDER_W, _BORDER_IDX):
            corner += wi * wj * (float(t2[i, j]) - float(p2[i, j]))
    mean_g = corner / (H * W)

    return np.asarray(0.2 * mean_g + 0.8 * mse, dtype=np.float32)


# revision 11
# speedup vs baseline: 1.0579x; 1.0169x over previous
"""Trainium2 Bass kernel for nn_ExperimentalLoss_23742579212660.

Loss = mean(0.2*G + 0.8*mse) where
  mse  = masked MSE over valid (target > 0) pixels,
  G    = blur3x3+sobel3x3(target) - blur3x3+sobel3x3(pred)  (reflect-101 pads).

Algebraic structure exploited:
  * mean(0.2*G + 0.8*mse) = 0.2*mean(G) + 0.8*mse.
  * The two stacked reflect-101 3x3 convs equal ONE separable 5-tap conv with
    c = [-1,-2,0,2,1]/4 per axis; sum(c)=0 makes the interior weight of
    sum(G) vanish, so mean(G) collapses to a fixed 36-term weighted sum of
    (target - pred) corner pixels, computed exactly on host from the f32
    inputs (~1e-8 here).
  * The memory-bound part is the masked MSE. The explicit 2e-2 error budget
    admits reduced input precision: inputs are rounded (RTNE) to fp8-e4m3 on
    host, quartering HBM traffic. Measured end-to-end effect on this input
    distribution: ~1.3e-5 relative (vs f32), 3 orders inside the gate.
  * Row-block sharded over 8 NeuronCores; each core streams its [512, 4096]
    slice (relaid out as [128, 16384]) of both tensors and emits [128, NJ]
    column partials of sum(mask*(t-p)^2) and sum(mask); host reduces in f64.

Device per tile [128, w] (one pass per engine, all tiles SBUF-resident):
  DVE : custom fused op  out = (t - p*(t>0))^2, accum -> sq col
        ( == mask*(t-p)^2 exactly, since t*mask == t )
  ACT : mask = Sign(t)   (t >= 0, so Sign == (t > 0)), accum -> count col
DMAs: t-loads ride the Sync HWDGE ring, p-loads the Scalar ring, both fully
issued up-front (every tile has its own buffer; fp8 tiles are tiny), so the
16 SDMA engines see both FIFO streams immediately and tile pairs land in
lockstep. One combined [128, 2*NJ] f32 result DMA at the end.
"""

import sys

import numpy as np

for _p in ("/opt/trn_rl_repo",):
    if _p not in sys.path:
        sys.path.insert(0, _p)

import ml_dtypes

H = 4096
W = 4096
N_CORES = 8
ROWS_PER_CORE = H // N_CORES          # 512
P = 128                               # SBUF partitions
COLS = ROWS_PER_CORE * W // P         # 16384 (per-core data as [128, 16384])
JOB_COLS = (512, 1024, 1536, 2560, 2560, 3584, 4608)
assert sum(JOB_COLS) == COLS
NJ = len(JOB_COLS)

HOST_DT = ml_dtypes.float8_e4m3       # matches device float8e4 decode

# Per-axis boundary weights of sum(G) (antisymmetric; interior weight is 0).
_BORDER_IDX = (0, 1, 2, H - 3, H - 2, H - 1)
_BORDER_W = (-0.75, -1.0, -0.25, 0.25, 1.0, 0.75)

_CACHED_NC = None


def _register_custom_op(name, spec):
    """Register a custom DVE op at runtime. The micro-op table is generated
    per-NEFF, so no firmware change is involved -- same mechanism as the
    production dve_ops.OPS entries."""
    import concourse.dve_ops as dve_ops
    from concourse.dve_spec import lower, _has_src1
    from concourse.dve_uop import DveOpSpec
    from concourse.dve_table_gen import dve_ver_for

    for op in dve_ops.OPS:
        if op.name == name:
            return op
    op = dve_ops.DveOp(name, spec, subdim=False, uops_sha={})
    dve_ops.OPS.append(op)
    dve_ops.CUSTOM_DVE_SPECS[name] = spec
    dve_ops._SUB_OPCODE_FOR_NAME[name] = (
        dve_ops._CUSTOM_DVE_ROW_BASE + len(dve_ops.OPS) - 1
    )
    ver = dve_ver_for("TRN2")
    dve_ops._COMPILE_CACHE[(name, ver)] = DveOpSpec(
        name=name,
        opcode=dve_ops.get_dve_sub_opcode(name),
        uops=lower(spec, ver=ver),
        rd1_en=_has_src1(spec),
    )
    return op


def _masked_sqdiff_op():
    """Fused DVE op: out = (in0 - in1*(in0>0))^2, accum_out = s0 + sum(out)."""
    from concourse.dve_spec import Spec, Src0, Src1, Zero, sq, C0
    from operator import add

    def _ref(in0, in1, s0, s1, imm2):
        m = (in0 > 0).astype(np.float32)
        b = ((in0.astype(np.float32) - in1 * m) ** 2).astype(np.float32)
        return b, s0 + b.reshape(b.shape[0], -1).sum(axis=-1, keepdims=True)

    return _register_custom_op(
        "MASKED_SQDIFF_LOSS_ANT",
        Spec(body=sq(Src0 - Src1 * (Src0 > Zero)), accum=add, accum_init=C0,
             reference=_ref),
    )


def _build_program():
    global _CACHED_NC
    if _CACHED_NC is not None:
        return _CACHED_NC

    from concourse import bacc, mybir
    import concourse.tile as tile

    f32 = mybir.dt.float32
    f8 = mybir.dt.float8e4
    AF = mybir.ActivationFunctionType
    msd_op = _masked_sqdiff_op()

    nc = bacc.Bacc(
        "TRN2",
        debug=False,
        target_bir_lowering=False,
        num_devices=N_CORES,
        enable_partition_id=False,
        enable_asserts=False,
    )
    t_d = nc.dram_tensor("t", [P, COLS], f8, kind="ExternalInput").ap()
    p_d = nc.dram_tensor("p", [P, COLS], f8, kind="ExternalInput").ap()
    out_d = nc.dram_tensor("o", [P, 2 * NJ], f32, kind="ExternalOutput").ap()

    col0 = [sum(JOB_COLS[:i]) for i in range(NJ)]

    with tile.TileContext(nc) as tc:
        with (
            tc.tile_pool(name="tin", bufs=1) as tpool,
            tc.tile_pool(name="pin", bufs=1) as ppool,
            tc.tile_pool(name="mask", bufs=2) as mpool,
            tc.tile_pool(name="dsq", bufs=2) as qpool,
            tc.tile_pool(name="acc", bufs=1) as apool,
        ):
            acc = apool.tile([P, 2 * NJ], f32, tag="acc")

            # All loads issued up-front; pair i rides ring i%2 (Sync/Scalar)
            # with t_i and p_i ADJACENT in the same queue, so FIFO order
            # guarantees each pair completes back-to-back (t/p on separate
            # rings skews pairs by several us; one ring for everything is
            # paced by the 4-deep HWDGE gen window at ~0.63us/DMA).  Every
            # tile has its own buffer (fp8 tiles are tiny), so neither ring
            # ever stalls on a buffer-release semaphore.
            tts, pts = [], []
            for i, w in enumerate(JOB_COLS):
                cs = slice(col0[i], col0[i] + w)
                ring = nc.sync if i % 2 == 0 else nc.scalar
                tt = tpool.tile([P, w], f8, tag=f"t_{w}_{i}", bufs=1)
                ring.dma_start(out=tt[:], in_=t_d[:, cs])
                tts.append(tt)
                pt = ppool.tile([P, w], f8, tag=f"p_{w}_{i}", bufs=1)
                ring.dma_start(out=pt[:], in_=p_d[:, cs])
                pts.append(pt)

            for i, w in enumerate(JOB_COLS):
                mask = mpool.tile([P, w], f8, tag="m", padded_shape=[P, 6144])
                nc.scalar.activation(
                    out=mask[:], in_=tts[i][:], func=AF.Sign,
                    accum_out=acc[:, NJ + i : NJ + i + 1],
                )
                dsq = qpool.tile([P, w], f8, tag="q", padded_shape=[P, 6144])
                nc.vector._custom_dve(
                    msd_op,
                    out=dsq[:], in0=tts[i][:], in1=pts[i][:],
                    s0=0.0, s1=0.0,
                    accum_out=acc[:, i : i + 1],
                )

            nc.sync.dma_start(out=out_d[:], in_=acc[:])

    nc.compile()
    _CACHED_NC = nc
    return nc


def _pack_cores(t2: np.ndarray, p2: np.ndarray):
    """Round both images to fp8 (RTNE) and lay each core's row block out as
    [128, 16384] (any bijective relayout is valid: the device only reduces)."""
    t8 = t2.astype(HOST_DT)
    p8 = p2.astype(HOST_DT)
    in_maps = []
    for c in range(N_CORES):
        rs = slice(c * ROWS_PER_CORE, (c + 1) * ROWS_PER_CORE)
        in_maps.append({
            "t": np.ascontiguousarray(t8[rs]).reshape(P, COLS),
            "p": np.ascontiguousarray(p8[rs]).reshape(P, COLS),
        })
    return in_maps


def _run_device(t2: np.ndarray, p2: np.ndarray, trace: bool = False):
    from concourse.bass_utils import run_bass_kernel_spmd

    nc = _build_program()
    in_maps = _pack_cores(t2, p2)
    return run_bass_kernel_spmd(nc, in_maps, list(range(N_CORES)), trace=trace)


def kernel(pred: np.ndarray, target: np.ndarray) -> np.ndarray:
    p2 = np.ascontiguousarray(np.asarray(pred, dtype=np.float32).reshape(H, W))
    t2 = np.ascontiguousarray(np.asarray(target, dtype=np.float32).reshape(H, W))

    results = _run_device(t2, p2).results

    S = 0.0
    C = 0.0
    for c in range(N_CORES):
        o = results[c]["o"].astype(np.float64)
        S += float(o[:, :NJ].sum())
        C += float(o[:, NJ:].sum())
    mse = S / max(C, 1.0)

    corner = 0.0
    for wi, i in zip(_BORDER_W, _BORDER_IDX):
        for wj, j in zip(_BORDER_W, _BORDER_IDX):
            corner += wi * wj * (float(t2[i, j]) - float(p2[i, j]))
    mean_g = corner / (H * W)

    return np.asarray(0.2 * mean_g + 0.8 * mse, dtype=np.float32)
